# revision 15
# baseline (speedup 1.0000x reference)
"""NonLocalAttention (embedded gaussian, no softmax) on 8 trn2 NeuronCores.

Reference math (per sample, all linear — no softmax):
    theta = conv1x1(a, theta_w, theta_b)        # [Ci, N]
    phi   = conv1x1(b, phi_w, phi_b)            # [Ci, N]
    g     = conv1x1(b, g_w, g_b)                # [Ci, N]
    f     = theta^T @ phi / N                   # [N, N]
    y     = f @ g^T                             # [N, Ci]
    out   = BN(W_w @ y^T)                       # [C, N]

Associativity rewrites (no nonlinearity anywhere between the matmuls):
  1. The NxN map is never materialized:
         MiT[ci2, ci1] = sum_m g[ci2, m] phi[ci1, m]            # [128, 128]
  2. W_w is folded into Mi before touching pixels again:
         P2T[ci1, c]   = sum_ci2 MiT[ci2, ci1] * W_w^T[ci2, c]  # [128, 256]
         out[c, n]     = BN( sum_ci1 P2T[ci1, c] theta[ci1, n] )
     so the y^T stage disappears entirely (P2 costs 256 PE cycles).

phi/g with pixels on partitions are produced DIRECTLY (no PE transposes):
the 128x128 b-tile is the stationary operand and the packed [phi|g] weight
pair streams 256 rows per matmul:
         pg_tile[m, 0:128]=phiT, [128:256]=gT = sum_c b[c, m] [phiT|gwT]
phi/g biases then cannot be folded into the (per-partition) eviction bias;
instead Mi gets an exact rank-2 correction computed on host from the column
sums of b:   MiT += rs_g0 pb^T + gb rs_phi0^T + N gb pb^T   (zero when the
biases are zero, but kept for full generality -- one 128x128 DVE add).

Everything on the wire is bf16 (DMA bytes halved; PE runs 1 cycle/row in
bf16 vs 4 for narrow-free f32r); all accumulation in f32 PSUM. A short PE
warmup chain on a zeroed tile ramps the tensor-engine pstate to 2.4 GHz
while the input DMAs stream. Emission order is the in-order PE schedule:
Mi matmuls lag their conv block by two so their ldweights (which carry the
eviction-semaphore waits) are always satisfied when they reach the queue
head, and the theta conv slots between the two b halves where `a` lands.

Sharding: 8 cores = 4 samples x 2 pixel-halves, zero inter-core traffic.
Core (s, h) loads the full b of sample s (phi/g/Mi are per-sample) and its
half of a, and produces output pixels [h*2048, (h+1)*2048).
"""

import numpy as np

B, C, Ci, H, W = 4, 256, 128, 64, 64
N_PIX = H * W            # 4096 pixels per sample
N_CORES = 8
HALF = N_PIX // 2        # 2048 output pixels per core
P = 128
CC = C // P              # 2 channel chunks
RB = 512                 # row block for conv/eviction batching
NB = N_PIX // RB         # 8 phi/g blocks over full b
TPB = RB // P            # 4 128-px tiles per block
QP = N_PIX // 4          # 1024-px DMA quarters of b
BN_EPS = 1e-5
N_WARM = 13              # PE warmup matmuls (256 rows each)
MI_LAG = 2               # conv blocks emitted ahead of their Mi matmuls

_CACHE = {}


def _build():
    import concourse.bacc as bacc
    import concourse.mybir as mybir
    import concourse.tile as tile

    f32 = mybir.dt.float32
    bf16 = mybir.dt.bfloat16
    Act = mybir.ActivationFunctionType
    Alu = mybir.AluOpType

    nc = bacc.Bacc("TRN2", num_devices=N_CORES)

    # packed weights: [thetaT/N (2x128) | [phiT|gwT] cc0 | [phiT|gwT] cc1 |
    #                  WT (256)]
    wpack_d = nc.dram_tensor("wpack", [P, 4 * C], bf16, kind="ExternalInput")
    # packed f32 per-partition cols: [tb/N | scale cc0,cc1 | shift cc0,cc1 |
    #                                 micorrT (128: rank-2 Mi bias correction)]
    vpack_d = nc.dram_tensor("vpack", [P, 5 + P], f32, kind="ExternalInput")
    a_d = nc.dram_tensor("a_half", [CC, P, HALF], bf16, kind="ExternalInput")
    b_d = nc.dram_tensor("b_full", [CC, P, N_PIX], bf16, kind="ExternalInput")
    out_d = nc.dram_tensor("out", [CC, P, HALF], bf16, kind="ExternalOutput")

    with tile.TileContext(nc) as tc:
        with (
            tc.tile_pool(name="const", bufs=1) as cpool,
            tc.tile_pool(name="big", bufs=1) as bpool,
            tc.tile_pool(name="work", bufs=3) as wpool,
            tc.tile_pool(name="ps", bufs=1, space="PSUM") as ppool,
        ):
            # ---- input DMAs first (transfers start ~1.2us in); order is the
            # critical path: b quarters 0-1, a (for theta mid-stream), b 2-3.
            wpack_sb = cpool.tile([P, 4 * C], bf16)
            vpack_sb = cpool.tile([P, 5 + P], f32)
            a_sb = bpool.tile([P, CC, HALF], bf16)
            b_sb = bpool.tile([P, CC, N_PIX], bf16)
            # memset first (Pool SEQ), then wpack rides SWDGE so its transfer
            # slots before the SP b stream; the conv/Mi phase is PE-bound and
            # must start ASAP (first b piece is a single 512-px block).
            # a lands mid-stream: theta merges into the PE-bound phase.
            zscr = cpool.tile([P, 2 * P], bf16)
            nc.gpsimd.memset(zscr[:], 0.0)
            nc.gpsimd.dma_start(out=wpack_sb[:], in_=wpack_d[:])
            edges = [0, RB, RB + QP, RB + 2 * QP, 3 * QP, N_PIX]
            for i, (p0, p1) in enumerate(zip(edges[:-1], edges[1:])):
                if i == 3:
                    for cc in range(CC):
                        nc.sync.dma_start(out=a_sb[:, cc, :], in_=a_d[cc, :, :])
                    nc.sync.dma_start(out=vpack_sb[:], in_=vpack_d[:])
                for cc in range(CC):
                    nc.sync.dma_start(
                        out=b_sb[:, cc, p0:p1], in_=b_d[cc, :, p0:p1]
                    )

            # ---- PE pstate warmup on the zeroed tile (no DMA dependency) --
            # touch the ACT function table now (1283ns load) so the first
            # real Identity activation doesn't pay it mid-phase
            actwarm = cpool.tile([P, 1], f32)
            nc.scalar.activation(actwarm[:], zscr[:, 0:1], Act.Identity, bias=0.0)
            warm_ps = ppool.tile([Ci, 2 * P], f32, tag="p2", bufs=1, name="warm")
            for w in range(N_WARM):
                nc.tensor.matmul(
                    warm_ps[:], zscr[:, :P], zscr[:],
                    start=(w == 0), stop=(w == N_WARM - 1),
                )

            thetaT_sb = wpack_sb[:, 0:C].rearrange("p (c k) -> p c k", c=CC)
            pg_w_sb = wpack_sb[:, C : 3 * C].rearrange("p (c k) -> p c k", c=CC)
            WT_sb = wpack_sb[:, 3 * C : 4 * C]
            tb_sb = vpack_sb[:, 0:1]
            scale_sb = vpack_sb[:, 1:3]
            shift_sb = vpack_sb[:, 3:5]
            micorr_sb = vpack_sb[:, 5 : 5 + P]

            theta_x = bpool.tile([Ci, HALF], bf16)
            miT_ps = ppool.tile([Ci, Ci], f32, tag="mi", bufs=1, name="miT_ps")

            # phi/g conv block k: two 128-px tiles' [phi|g] pairs accumulate
            # per [128, 512] PSUM bank; b-tiles stationary, weights moving.
            def conv_block(k):
                sbufs = []
                for h in range(2):
                    ps = ppool.tile([P, RB], f32, tag="mm512", bufs=6, name="pg")
                    for t in (2 * h, 2 * h + 1):
                        m0 = k * RB + t * P
                        csl = slice((t % 2) * 2 * P, ((t % 2) + 1) * 2 * P)
                        for cc in range(CC):
                            nc.tensor.matmul(
                                ps[:, csl],
                                b_sb[:, cc, m0 : m0 + P],
                                pg_w_sb[:, cc, :],
                                start=(cc == 0), stop=(cc == CC - 1),
                            )
                    sb = wpool.tile([P, RB], bf16, tag=f"pg{h}", bufs=3,
                                    name=f"pg{h}")
                    if h == 0:
                        nc.scalar.activation(sb[:], ps[:], Act.Copy)
                    else:
                        nc.vector.tensor_copy(sb[:], ps[:])
                    sbufs.append(sb)
                return sbufs

            def mi_block(k, blk):
                for t in range(TPB):
                    sb = blk[t // 2]
                    base = (t % 2) * 2 * P
                    nc.tensor.matmul(
                        miT_ps[:],
                        sb[:, base + P : base + 2 * P],   # gT tile (stationary)
                        sb[:, base : base + P],           # phiT tile (moving)
                        start=(k == 0 and t == 0),
                        stop=(k == NB - 1 and t == TPB - 1),
                    )

            def theta_block(r):
                rows = slice(r * RB, (r + 1) * RB)
                th_ps = ppool.tile([Ci, RB], f32, tag="mm512", bufs=6, name="th")
                for cc in range(CC):
                    nc.tensor.matmul(
                        th_ps[:], thetaT_sb[:, cc, :], a_sb[:, cc, rows],
                        start=(cc == 0), stop=(cc == CC - 1),
                    )
                # alternate eviction engines: ACT also carries the phi-half
                # pg evictions in this window
                if r % 2 == 0:
                    nc.scalar.activation(
                        theta_x[:, rows], th_ps[:], Act.Identity, bias=tb_sb
                    )
                else:
                    nc.vector.tensor_scalar_add(theta_x[:, rows], th_ps[:], tb_sb)

            blks = {}
            for k in range(NB):
                if k == 5:
                    for r in range(HALF // RB):
                        theta_block(r)
                blks[k] = conv_block(k)
                if k - MI_LAG >= 0:
                    mi_block(k - MI_LAG, blks.pop(k - MI_LAG))
            for k in sorted(blks):
                mi_block(k, blks.pop(k))

            # ---- MiT (+ bias correction) -> P2T = MiT^T-contract W^T -------
            miT_sb = wpool.tile([Ci, Ci], bf16, tag="miT", bufs=1, name="miT_sb")
            nc.vector.tensor_add(miT_sb[:], miT_ps[:], micorr_sb)
            p2_ps = ppool.tile([Ci, C], f32, tag="p2", bufs=1, name="p2_ps")
            nc.tensor.matmul(p2_ps[:], miT_sb[:], WT_sb, start=True, stop=True)
            p2T_sb = wpool.tile([Ci, C], bf16, tag="p2sb", bufs=1, name="p2T_sb")
            nc.vector.tensor_copy(p2T_sb[:], p2_ps[:])

            # ---- out[c, n] = BN( P2T^T @ theta_x ); store per 512-block ----
            for r in range(HALF // RB):
                rows = slice(r * RB, (r + 1) * RB)
                osb = wpool.tile([P, CC, RB], bf16, tag="osb", bufs=4, name="osb")
                wy_ps = []
                for cc in range(CC):
                    ps = ppool.tile([P, RB], f32, tag="mm512", bufs=6, name="wy")
                    nc.tensor.matmul(
                        ps[:], p2T_sb[:, cc * P : (cc + 1) * P], theta_x[:, rows],
                        start=True, stop=True,
                    )
                    wy_ps.append(ps)
                # BN epilogues split across ACT (cc0) and DVE (cc1)
                nc.scalar.activation(
                    osb[:, 0, :], wy_ps[0][:], Act.Identity,
                    bias=shift_sb[:, 0:1], scale=scale_sb[:, 0:1],
                )
                nc.vector.tensor_scalar(
                    osb[:, 1, :], wy_ps[1][:],
                    scale_sb[:, 1:2], shift_sb[:, 1:2], Alu.mult, Alu.add,
                )
                nc.sync.dma_start(
                    out=out_d[:, :, rows].rearrange("c p r -> p c r"), in_=osb[:]
                )

    nc.compile()
    return nc


def _get_nc():
    if "nc" not in _CACHE:
        _CACHE["nc"] = _build()
    return _CACHE["nc"]


def _prep_in_maps(a, b, theta_w, theta_b, phi_w, phi_b, g_w, g_b, W_w,
                  bn_gamma, bn_beta, bn_mean, bn_var):
    import ml_dtypes

    f = np.float32
    bf = ml_dtypes.bfloat16
    a4 = np.asarray(a, f).reshape(B, C, N_PIX).astype(bf)
    b4 = np.asarray(b, f).reshape(B, C, N_PIX).astype(bf)

    inv_n = 1.0 / np.float64(N_PIX)
    thetaT = (np.asarray(theta_w, f).T * inv_n).astype(f)   # [C, Ci]
    phiT = np.asarray(phi_w, f).T                           # [C, Ci]
    gwT = np.asarray(g_w, f).T                              # [C, Ci]
    WT = np.asarray(W_w, f).T                               # [Ci, C]
    wpack = np.empty((P, 4 * C), f)
    for cc in range(CC):
        wpack[:, cc * Ci : (cc + 1) * Ci] = thetaT[cc * P : (cc + 1) * P, :]
        base = C + cc * 2 * Ci
        wpack[:, base : base + Ci] = phiT[cc * P : (cc + 1) * P, :]
        wpack[:, base + Ci : base + 2 * Ci] = gwT[cc * P : (cc + 1) * P, :]
    wpack[:, 3 * C : 4 * C] = WT
    wpack = np.ascontiguousarray(wpack.astype(bf))

    scale = (np.asarray(bn_gamma, f) / np.sqrt(np.asarray(bn_var, f) + BN_EPS)).astype(f)
    shift = (np.asarray(bn_beta, f) - np.asarray(bn_mean, f) * scale).astype(f)

    # Rank-2 Mi bias correction, exact given column sums of b (host f64):
    # MiT_corr = rs_g0 pb^T + gb rs_phi0^T + N gb pb^T
    pb = np.asarray(phi_b, np.float64)
    gb = np.asarray(g_b, np.float64)
    b_f64 = np.asarray(b, np.float64).reshape(B, C, N_PIX)
    rs_b = b_f64.sum(axis=2)                                # [B, C]
    rs_phi0 = rs_b @ np.asarray(phi_w, np.float64).T        # [B, Ci]
    rs_g0 = rs_b @ np.asarray(g_w, np.float64).T            # [B, Ci]

    in_maps = []
    for core in range(N_CORES):
        s, h = divmod(core, 2)
        micorrT = (
            np.outer(rs_g0[s], pb) + np.outer(gb, rs_phi0[s])
            + N_PIX * np.outer(gb, pb)
        ).astype(f)                                         # [ci2, ci1]
        vpack = np.concatenate(
            [
                (np.asarray(theta_b, f) * inv_n)[:, None].astype(f),
                scale.reshape(CC, P).T,
                shift.reshape(CC, P).T,
                micorrT,
            ],
            axis=1,
        )
        in_maps.append({
            "a_half": np.ascontiguousarray(
                a4[s][:, h * HALF : (h + 1) * HALF].reshape(CC, P, HALF)),
            "b_full": np.ascontiguousarray(b4[s].reshape(CC, P, N_PIX)),
            "wpack": wpack,
            "vpack": np.ascontiguousarray(vpack),
        })
    return in_maps


def run(inputs: dict, trace: bool = False):
    from concourse.bass_utils import run_bass_kernel_spmd

    nc = _get_nc()
    in_maps = _prep_in_maps(**inputs)
    res = run_bass_kernel_spmd(nc, in_maps, list(range(N_CORES)), trace=trace)
    out = np.empty((B, C, N_PIX), np.float32)
    for core in range(N_CORES):
        s, h = divmod(core, 2)
        out[s][:, h * HALF : (h + 1) * HALF] = \
            np.asarray(res.results[core]["out"], np.float32).reshape(C, HALF)
    return out.reshape(B, C, H, W), res


def kernel(**inputs) -> np.ndarray:
    out, _ = run(inputs, trace=False)
    return out
